# revision 4
# baseline (speedup 1.0000x reference)
"""Trainium2 Bass kernel for the DU-LM-SB (simulated bifurcation MIMO detector) problem.

Contract: kernel(**inputs) takes the FULL unsharded inputs (see reference
setup_inputs) and returns the full (B, Ns) spin output.  Internally the
batch dim B=16384 is sharded over 8 NeuronCores (2048 per core); J/h and
all per-step scalars are replicated.  The T-step scan runs fully on-device
out of SBUF; only the initial state/J load and final state store touch HBM.
"""
import math
import sys

for _p in ("/root/.axon_site", "/root/.axon_site/_ro/trn_rl_repo", "/opt/trn_rl_repo"):
    if _p not in sys.path:
        sys.path.append(_p)

import numpy as np
from contextlib import ExitStack

import concourse.bacc as bacc
import concourse.tile as tile
from concourse import mybir
from concourse.bass_utils import run_bass_kernel_spmd

F32 = mybir.dt.float32
F32R = mybir.dt.float32r
I32 = mybir.dt.int32
AL = mybir.AluOpType
AF = mybir.ActivationFunctionType

NCORES = 8
NS = 768          # spin dim (= 2*Nt*rb)
MT = NS // 128    # 6 row tiles of 128
BP = 16384 // NCORES  # batch per core (2048)
NCH = 512         # free-dim chunk for matmul/elementwise
NCHUNKS = BP // NCH


def _build_nc(A, B, G, T):
    """Build + compile the per-core program. A,B,G are length-T python float lists."""
    nc = bacc.Bacc("TRN2", target_bir_lowering=False, debug=False, num_devices=NCORES)

    j_in = nc.dram_tensor("j", [NS, NS], F32, kind="ExternalInput").ap()
    hb_in = nc.dram_tensor("hb", [NS, T], F32, kind="ExternalInput").ap()
    x0_in = nc.dram_tensor("x0", [NS, BP], F32, kind="ExternalInput").ap()
    w0_in = nc.dram_tensor("w0", [NS, BP], F32, kind="ExternalInput").ap()
    xout = nc.dram_tensor("xout", [NS, BP], F32, kind="ExternalOutput").ap()

    with tile.TileContext(nc) as tc, ExitStack() as ctx:
        pj = ctx.enter_context(tc.tile_pool(name="pj", bufs=1))
        pstate = ctx.enter_context(tc.tile_pool(name="pstate", bufs=1))
        ptmp = ctx.enter_context(tc.tile_pool(name="ptmp", bufs=3))
        pps = ctx.enter_context(tc.tile_pool(name="pps", bufs=2, space="PSUM"))

        # --- static data ---
        jr = [pj.tile([128, NS], F32R, name=f"jr{k}") for k in range(MT)]
        hbt = [pj.tile([128, T], F32, name=f"hbt{m}") for m in range(MT)]
        cm1 = pj.tile([128, 1], F32)
        nc.vector.memset(cm1[:], -1.0)
        c505 = pj.tile([128, 1], F32)
        nc.vector.memset(c505[:], 50.5)
        # --- state (x double-buffered in f32r, w in f32) ---
        xa = [pstate.tile([128, BP], F32R, name=f"xa{m}") for m in range(MT)]
        xb = [pstate.tile([128, BP], F32R, name=f"xb{m}") for m in range(MT)]
        wt = [pstate.tile([128, BP], F32, name=f"wt{m}") for m in range(MT)]
        for k in range(MT):
            # bounce J rows through wt[k] (later overwritten by w0) to round into f32r
            nc.sync.dma_start(wt[k][:, 0:NS], j_in[k * 128:(k + 1) * 128, :])
            nc.vector.tensor_copy(jr[k][:], wt[k][:, 0:NS])
            nc.sync.dma_start(hbt[k][:], hb_in[k * 128:(k + 1) * 128, :])
        for m in range(MT):
            # bounce x0 through wt[m] (f32) to round into f32r, then load w0
            nc.sync.dma_start(wt[m][:], x0_in[m * 128:(m + 1) * 128, :])
            nc.vector.tensor_copy(xa[m][:], wt[m][:])
            nc.sync.dma_start(wt[m][:], w0_in[m * 128:(m + 1) * 128, :])

        # --- the scan ---
        for k in range(T):
            xc = xa if k % 2 == 0 else xb
            xn = xb if k % 2 == 0 else xa
            Ak, Bk, Gk = A[k], B[k], G[k]
            for m in range(MT):
                psum = pps.tile([128, BP], F32, name="ps", tag="ps")
                for kk in range(MT):
                    for c in range(NCHUNKS):
                        nc.tensor.matmul(
                            psum[:, c * NCH:(c + 1) * NCH],
                            jr[kk][:, m * 128:(m + 1) * 128],
                            xc[kk][:, c * NCH:(c + 1) * NCH],
                            start=(kk == 0), stop=(kk == MT - 1),
                        )
                for c in range(NCHUNKS):
                    cs = slice(c * NCH, (c + 1) * NCH)
                    vt = ptmp.tile([128, NCH], F32, name="vt", tag="vt")
                    sa = ptmp.tile([128, NCH], F32, name="sa", tag="sa")
                    sb_ = ptmp.tile([128, NCH], F32, name="sb_", tag="sb_")
                    # v = A*psum + hb  (ACT), then += B*x, then += G*w  (DVE STT)
                    nc.scalar.activation(vt[:], psum[:, cs], AF.Identity,
                                         bias=hbt[m][:, k:k + 1], scale=Ak)
                    nc.vector.scalar_tensor_tensor(vt[:], xc[m][:, cs].bitcast(F32), Bk,
                                                   vt[:], AL.mult, AL.add)
                    nc.vector.scalar_tensor_tensor(vt[:], wt[m][:, cs], Gk,
                                                   vt[:], AL.mult, AL.add)
                    # x' = silu(v+1) - silu(v-1) - 1  -> f32r state
                    nc.scalar.activation(sa[:], vt[:], AF.Silu, bias=1.0, scale=1.0)
                    nc.scalar.activation(sb_[:], vt[:], AF.Silu, bias=cm1[:], scale=1.0)
                    nc.vector.scalar_tensor_tensor(xn[m][:, cs], sa[:], 1.0, sb_[:],
                                                   AL.subtract, AL.subtract)
                    # mask: t = tanh(-50*|x'| + 50.5); w' = (1+t)*(v - x)
                    nc.vector.tensor_scalar(sa[:].bitcast(I32), xn[m][:, cs].bitcast(I32),
                                            0x7FFFFFFF, None, AL.bitwise_and)
                    nc.gpsimd.tensor_tensor(sb_[:], vt[:], xc[m][:, cs].bitcast(F32),
                                            AL.subtract)
                    nc.scalar.activation(sa[:], sa[:], AF.Tanh, bias=c505[:], scale=-50.0)
                    nc.vector.scalar_tensor_tensor(wt[m][:, cs], sa[:], 1.0, sb_[:],
                                                   AL.add, AL.mult)

        xf = xa if T % 2 == 0 else xb
        for m in range(MT):
            nc.sync.dma_start(xout[m * 128:(m + 1) * 128, :], xf[m][:].bitcast(F32))

    nc.compile()
    return nc


def _host_precompute(H_real, H_imag, y_real, y_imag, delta, eta, lam, nbps):
    Hr = np.asarray(H_real, np.float64)
    Hi = np.asarray(H_imag, np.float64)
    yr = np.asarray(y_real, np.float64)
    yi = np.asarray(y_imag, np.float64)
    d = np.asarray(delta, np.float64)
    eta_s = float(np.asarray(eta).reshape(-1)[0])
    lam_s = float(np.asarray(lam).reshape(-1)[0])
    nbps = int(nbps)
    M = 2 ** nbps
    Nr, Nt = Hr.shape
    N = 2 * Nt
    rb = nbps // 2
    qam_var = 2.0 * (M - 1) / 3.0
    I = np.eye(N)
    powers = 2.0 ** (rb - 1 - np.arange(rb))
    Tm = (powers[:, None, None] * I[None, :, :]).reshape(-1, N).T
    H_t = np.block([[Hr, -Hi], [Hi, Hr]])
    y_t = np.concatenate([yr, yi], axis=0)
    U = np.linalg.inv(H_t @ H_t.T + lam_s * I) / lam_s
    HT = H_t @ Tm
    J = -(HT.T @ U @ HT) * (2.0 / qam_var)
    J = J * (1.0 - np.eye(J.shape[0]))
    z = (y_t - HT @ np.ones((N * rb, 1)) + (math.sqrt(M) - 1.0) * (H_t @ np.ones((N, 1)))) / math.sqrt(qam_var)
    h = (2.0 * (HT.T @ (U @ z)))[:, 0]
    T = d.shape[0]
    Ns = J.shape[0]
    a = np.linspace(0.0, 1.0, T)
    c0 = 2.0 * math.sqrt((Ns - 1) / float(np.sum(J * J)))
    A = [float(d[k] * d[k] * eta_s * c0) for k in range(T)]
    B = [float(1.0 - d[k] * d[k] * (1.0 - a[k])) for k in range(T)]
    G = [float(d[0])] + [float(d[k] / (2.0 * d[k - 1])) for k in range(1, T)]
    HB = np.empty((Ns, T), np.float32)
    for k in range(T):
        HB[:, k] = (A[k] * h).astype(np.float32)
    return J.astype(np.float32), h, A, B, G, HB, T


def kernel(H_real, H_imag, y_real, y_imag, delta, eta, lam, x0, y0, nbps, _T=None):
    J32, h, A, B, G, HB, T = _host_precompute(
        H_real, H_imag, y_real, y_imag, delta, eta, lam, nbps)
    if _T is not None:
        T = _T
    X0 = (0.02 * (np.asarray(x0, np.float64) - 0.5)).astype(np.float32)
    W0 = (0.02 * (np.asarray(y0, np.float64) - 0.5)).astype(np.float32)

    nc = _build_nc(A[:T], B[:T], G[:T], T)
    in_maps = []
    for i in range(NCORES):
        s = slice(i * BP, (i + 1) * BP)
        in_maps.append({"j": J32, "hb": np.ascontiguousarray(HB[:, :T]),
                        "x0": np.ascontiguousarray(X0[:, s]),
                        "w0": np.ascontiguousarray(W0[:, s])})
    res = run_bass_kernel_spmd(nc, in_maps, list(range(NCORES)))
    global LAST_RESULTS
    LAST_RESULTS = res
    out = np.concatenate([res.results[i]["xout"] for i in range(NCORES)], axis=1)
    return np.ascontiguousarray(out.T)


LAST_RESULTS = None


# revision 5
# speedup vs baseline: 1.2128x; 1.2128x over previous
"""Trainium2 Bass kernel for the DU-LM-SB (simulated bifurcation MIMO detector) problem.

Contract: kernel(**inputs) takes the FULL unsharded inputs (see reference
setup_inputs) and returns the full (B, Ns) spin output.  Internally the
batch dim B=16384 is sharded over 8 NeuronCores (2048 per core); J/h and
all per-step scalars are replicated.  The T-step scan runs fully on-device
out of SBUF; HBM traffic is the initial state load, a per-step stream of
the folded coupling matrix J~_k = A_k*J + B_k*I, and the final state store.

Recurrence: with state X_k = x_k and W_k the unnormalized masked momentum
(y entering step k equals gamma_k * W_k), one step is
    psum = J~_k @ X_k + A_k*h          (PE, fp32r; bias via ones-row matmul)
    v    = G_k*W_k + psum              (DVE scalar_tensor_tensor)
    X'   = silu(v+1) - silu(v-1) - 1   (ACT Silu x2 + DVE STT, fp32r out)
    t    = tanh(-50*|X'| + 50.5)       (DVE int-AND abs + ACT Tanh)
    W'   = (1+t) * (v - X_k)           (GPSIMD sub + DVE STT)
"""
import math
import sys

for _p in ("/root/.axon_site", "/root/.axon_site/_ro/trn_rl_repo", "/opt/trn_rl_repo"):
    if _p not in sys.path:
        sys.path.append(_p)

import numpy as np
from contextlib import ExitStack

import concourse.bacc as bacc
import concourse.tile as tile
from concourse import mybir
from concourse.bass_utils import run_bass_kernel_spmd

F32 = mybir.dt.float32
F32R = mybir.dt.float32r
I32 = mybir.dt.int32
AL = mybir.AluOpType
AF = mybir.ActivationFunctionType

NCORES = 8
NS = 768              # spin dim (= 2*Nt*rb)
MT = NS // 128        # 6 row tiles of 128
BP = 16384 // NCORES  # batch per core (2048)
NCH = 512             # matmul moving-dim chunk (fp32 limit)
NCHUNKS = BP // NCH
ECH = 1024            # elementwise chunk
ECHUNKS = BP // ECH


def _build_nc(G, T):
    """Build + compile the per-core program. G: length-T python float list."""
    nc = bacc.Bacc("TRN2", target_bir_lowering=False, debug=False, num_devices=NCORES)

    jt_in = nc.dram_tensor("jt", [T * NS, NS], F32R, kind="ExternalInput").ap()
    hrow_in = nc.dram_tensor("hrow", [1, NS], F32R, kind="ExternalInput").ap()
    av_in = nc.dram_tensor("avals", [1, T], F32R, kind="ExternalInput").ap()
    x0_in = nc.dram_tensor("x0", [NS, BP], F32R, kind="ExternalInput").ap()
    w0_in = nc.dram_tensor("w0", [NS, BP], F32, kind="ExternalInput").ap()
    xout = nc.dram_tensor("xout", [NS, BP], F32, kind="ExternalOutput").ap()

    with tile.TileContext(nc) as tc, ExitStack() as ctx:
        pj = ctx.enter_context(tc.tile_pool(name="pj", bufs=1))
        pstate = ctx.enter_context(tc.tile_pool(name="pstate", bufs=1))
        ptmp = ctx.enter_context(tc.tile_pool(name="ptmp", bufs=2))
        pps = ctx.enter_context(tc.tile_pool(name="pps", bufs=2, space="PSUM"))

        # --- static data ---
        jrr = [pj.tile([128, NS], F32R, name=f"jrr{k}") for k in range(MT)]
        hrow = pj.tile([1, NS], F32R)
        av = pj.tile([1, T], F32R)
        cm1 = pj.tile([128, 1], F32)
        c505 = pj.tile([128, 1], F32)
        nc.vector.memset(cm1[:], -1.0)
        nc.vector.memset(c505[:], 50.5)
        nc.sync.dma_start(hrow[:], hrow_in)
        nc.sync.dma_start(av[:], av_in)

        # --- state (x double-buffered in f32r, w in f32) ---
        xa = [pstate.tile([128, BP], F32R, name=f"xa{m}") for m in range(MT)]
        xb = [pstate.tile([128, BP], F32R, name=f"xb{m}") for m in range(MT)]
        wt = [pstate.tile([128, BP], F32, name=f"wt{m}") for m in range(MT)]
        for m in range(MT):
            nc.sync.dma_start(xa[m][:], x0_in[m * 128:(m + 1) * 128, :])
            nc.sync.dma_start(wt[m][:], w0_in[m * 128:(m + 1) * 128, :])

        # --- the scan ---
        for k in range(T):
            xc = xa if k % 2 == 0 else xb
            xn = xb if k % 2 == 0 else xa
            Gk = G[k]
            base = k * NS
            for kk in range(MT):
                nc.sync.dma_start(jrr[kk][:], jt_in[base + kk * 128: base + (kk + 1) * 128, :])
            for m in range(MT):
                psum = pps.tile([128, BP], mybir.dt.float32, name="ps", tag="ps")
                msl = slice(m * 128, (m + 1) * 128)
                for kk in range(MT):
                    for c in range(NCHUNKS):
                        nc.tensor.matmul(
                            psum[:, c * NCH:(c + 1) * NCH],
                            jrr[kk][:, msl],
                            xc[kk][:, c * NCH:(c + 1) * NCH],
                            start=(kk == 0), stop=False,
                        )
                for c in range(NCHUNKS):
                    # bias: psum += h[m-block]^T @ (A_k broadcast row)
                    nc.tensor.matmul(
                        psum[:, c * NCH:(c + 1) * NCH],
                        hrow[0:1, msl],
                        av[0:1, k:k + 1].to_broadcast([1, NCH]),
                        start=False, stop=True,
                    )
                for c in range(ECHUNKS):
                    cs = slice(c * ECH, (c + 1) * ECH)
                    vt = ptmp.tile([128, ECH], F32, name="vt", tag="vt")
                    sa = ptmp.tile([128, ECH], F32, name="sa", tag="sa")
                    sb_ = ptmp.tile([128, ECH], F32, name="sb_", tag="sb_")
                    # v = G*w + psum
                    nc.vector.scalar_tensor_tensor(vt[:], wt[m][:, cs], Gk,
                                                   psum[:, cs], AL.mult, AL.add)
                    # x' = silu(v+1) - silu(v-1) - 1  -> f32r state
                    nc.scalar.activation(sa[:], vt[:], AF.Silu, bias=1.0, scale=1.0)
                    nc.scalar.activation(sb_[:], vt[:], AF.Silu, bias=cm1[:], scale=1.0)
                    nc.vector.scalar_tensor_tensor(xn[m][:, cs], sa[:], 1.0, sb_[:],
                                                   AL.subtract, AL.subtract)
                    # t = tanh(-50|x'| + 50.5)
                    nc.vector.tensor_scalar(sa[:].bitcast(I32), xn[m][:, cs].bitcast(I32),
                                            0x7FFFFFFF, None, AL.bitwise_and)
                    nc.gpsimd.tensor_tensor(sb_[:], vt[:], xc[m][:, cs].bitcast(F32),
                                            AL.subtract)
                    nc.scalar.activation(sa[:], sa[:], AF.Tanh, bias=c505[:], scale=-50.0)
                    # w' = (1+t) * (v - x)
                    nc.vector.scalar_tensor_tensor(wt[m][:, cs], sa[:], 1.0, sb_[:],
                                                   AL.add, AL.mult)

        xf = xa if T % 2 == 0 else xb
        for m in range(MT):
            nc.sync.dma_start(xout[m * 128:(m + 1) * 128, :], xf[m][:].bitcast(F32))

    nc.compile()
    return nc


def _host_precompute(H_real, H_imag, y_real, y_imag, delta, eta, lam, nbps):
    Hr = np.asarray(H_real, np.float64)
    Hi = np.asarray(H_imag, np.float64)
    yr = np.asarray(y_real, np.float64)
    yi = np.asarray(y_imag, np.float64)
    d = np.asarray(delta, np.float64)
    eta_s = float(np.asarray(eta).reshape(-1)[0])
    lam_s = float(np.asarray(lam).reshape(-1)[0])
    nbps = int(nbps)
    M = 2 ** nbps
    Nr, Nt = Hr.shape
    N = 2 * Nt
    rb = nbps // 2
    qam_var = 2.0 * (M - 1) / 3.0
    I = np.eye(N)
    powers = 2.0 ** (rb - 1 - np.arange(rb))
    Tm = (powers[:, None, None] * I[None, :, :]).reshape(-1, N).T
    H_t = np.block([[Hr, -Hi], [Hi, Hr]])
    y_t = np.concatenate([yr, yi], axis=0)
    U = np.linalg.inv(H_t @ H_t.T + lam_s * I) / lam_s
    HT = H_t @ Tm
    J = -(HT.T @ U @ HT) * (2.0 / qam_var)
    J = J * (1.0 - np.eye(J.shape[0]))
    z = (y_t - HT @ np.ones((N * rb, 1)) + (math.sqrt(M) - 1.0) * (H_t @ np.ones((N, 1)))) / math.sqrt(qam_var)
    h = (2.0 * (HT.T @ (U @ z)))[:, 0]
    T = d.shape[0]
    Ns = J.shape[0]
    a = np.linspace(0.0, 1.0, T)
    c0 = 2.0 * math.sqrt((Ns - 1) / float(np.sum(J * J)))
    # x_pre = B_k x + A_k (Jx) + A_k h + (d_k gamma_k) W
    A = [float(d[k] * d[k] * eta_s * c0) for k in range(T)]
    B = [float(1.0 - d[k] * d[k] * (1.0 - a[k])) for k in range(T)]
    G = [float(d[0])] + [float(d[k] / (2.0 * d[k - 1])) for k in range(1, T)]
    return J, h, A, B, G, T


def kernel(H_real, H_imag, y_real, y_imag, delta, eta, lam, x0, y0, nbps, _T=None):
    J, h, A, B, G, T = _host_precompute(
        H_real, H_imag, y_real, y_imag, delta, eta, lam, nbps)
    if _T is not None:
        T = _T
    X0 = (0.02 * (np.asarray(x0, np.float64) - 0.5)).astype(np.float32)
    W0 = (0.02 * (np.asarray(y0, np.float64) - 0.5)).astype(np.float32)

    # stacked folded matrices: J~_k = A_k*J + B_k*I  (T*NS x NS, fp32)
    J32 = J.astype(np.float32)
    JT = np.empty((T * NS, NS), np.float32)
    eye = np.eye(NS, dtype=np.float32)
    for k in range(T):
        JT[k * NS:(k + 1) * NS] = np.float32(A[k]) * J32 + np.float32(B[k]) * eye
    hrow = h.astype(np.float32).reshape(1, NS)
    avals = np.asarray(A, np.float32).reshape(1, -1)

    nc = _build_nc(G[:T], T)
    in_maps = []
    for i in range(NCORES):
        s = slice(i * BP, (i + 1) * BP)
        in_maps.append({"jt": JT, "hrow": hrow, "avals": avals[:, :T],
                        "x0": np.ascontiguousarray(X0[:, s]),
                        "w0": np.ascontiguousarray(W0[:, s])})
    res = run_bass_kernel_spmd(nc, in_maps, list(range(NCORES)))
    global LAST_RESULTS
    LAST_RESULTS = res
    out = np.concatenate([res.results[i]["xout"] for i in range(NCORES)], axis=1)
    return np.ascontiguousarray(out.T)


LAST_RESULTS = None


# revision 8
# speedup vs baseline: 1.3900x; 1.1461x over previous
"""Trainium2 Bass kernel for the DU-LM-SB (simulated bifurcation MIMO detector) problem.

Contract: kernel(**inputs) takes the FULL unsharded inputs (see reference
setup_inputs) and returns the full (B, Ns) spin output.  Internally the
batch dim B=16384 is sharded over 8 NeuronCores (2048 per core); J/h and
all per-step scalars are replicated.  The T-step scan runs fully on-device
out of SBUF; HBM traffic is the initial state load, a per-step stream of
the folded coupling matrix J~_k = A_k*J + B_k*I, and the final state store.

Recurrence: with state X_k = x_k and W_k the unnormalized masked momentum
(y entering step k equals gamma_k * W_k), one step is
    psum = J~_k @ X_k + A_k*h          (PE, fp32r; bias via ones-row matmul)
    v    = G_k*W_k + psum              (DVE scalar_tensor_tensor)
    X'   = silu(v+1) - silu(v-1) - 1   (ACT Silu x2 + DVE STT, fp32r out)
    t    = tanh(-50*|X'| + 50.5)       (DVE int-AND abs + ACT Tanh)
    W'   = (1+t) * (v - X_k)           (GPSIMD sub + DVE STT)
"""
import math
import sys

for _p in ("/root/.axon_site", "/root/.axon_site/_ro/trn_rl_repo", "/opt/trn_rl_repo"):
    if _p not in sys.path:
        sys.path.append(_p)

import numpy as np
from contextlib import ExitStack

import concourse.bacc as bacc
import concourse.tile as tile
from concourse import mybir
from concourse.bass_utils import run_bass_kernel_spmd

F32 = mybir.dt.float32
F32R = mybir.dt.float32r
I32 = mybir.dt.int32
AL = mybir.AluOpType
AF = mybir.ActivationFunctionType

NCORES = 8
NS = 768              # spin dim (= 2*Nt*rb)
MT = NS // 128        # 6 row tiles of 128
BP = 16384 // NCORES  # batch per core (2048)
NCH = 512             # matmul moving-dim chunk (fp32 limit)
NCHUNKS = BP // NCH
ECH = 1024            # elementwise chunk
ECHUNKS = BP // ECH


def _build_nc(G, T):
    """Build + compile the per-core program. G: length-T python float list."""
    nc = bacc.Bacc("TRN2", target_bir_lowering=False, debug=False, num_devices=NCORES)

    jt_in = nc.dram_tensor("jt", [T * NS, NS], F32R, kind="ExternalInput").ap()
    hrow_in = nc.dram_tensor("hrow", [1, NS], F32R, kind="ExternalInput").ap()
    av_in = nc.dram_tensor("avals", [1, T], F32R, kind="ExternalInput").ap()
    x0_in = nc.dram_tensor("x0", [NS, BP], F32R, kind="ExternalInput").ap()
    w0_in = nc.dram_tensor("w0", [NS, BP], F32, kind="ExternalInput").ap()
    xout = nc.dram_tensor("xout", [NS, BP], F32, kind="ExternalOutput").ap()

    with tile.TileContext(nc) as tc, ExitStack() as ctx:
        pj = ctx.enter_context(tc.tile_pool(name="pj", bufs=1))
        pstate = ctx.enter_context(tc.tile_pool(name="pstate", bufs=1))
        ptmp = ctx.enter_context(tc.tile_pool(name="ptmp", bufs=4))
        ptmp3 = ctx.enter_context(tc.tile_pool(name="ptmp3", bufs=3))
        pps = ctx.enter_context(tc.tile_pool(name="pps", bufs=4, space="PSUM"))

        # --- static data ---
        jrr = [pj.tile([128, NS], F32R, name=f"jrr{k}") for k in range(MT)]
        hrow = pj.tile([1, NS], F32R)
        av = pj.tile([1, T], F32R)
        cm1 = pj.tile([128, 1], F32)
        c505 = pj.tile([128, 1], F32)
        nc.vector.memset(cm1[:], -1.0)
        nc.vector.memset(c505[:], 50.5)
        nc.sync.dma_start(hrow[:], hrow_in)
        nc.sync.dma_start(av[:], av_in)

        # --- state (x double-buffered in f32r, w in f32) ---
        xa = [pstate.tile([128, BP], F32R, name=f"xa{m}") for m in range(MT)]
        xb = [pstate.tile([128, BP], F32R, name=f"xb{m}") for m in range(MT)]
        wt = [pstate.tile([128, BP], F32, name=f"wt{m}") for m in range(MT)]
        for m in range(MT):
            nc.sync.dma_start(xa[m][:], x0_in[m * 128:(m + 1) * 128, :])
            nc.sync.dma_start(wt[m][:], w0_in[m * 128:(m + 1) * 128, :])

        # --- the scan ---
        for k in range(T):
            xc = xa if k % 2 == 0 else xb
            xn = xb if k % 2 == 0 else xa
            Gk = G[k]
            base = k * NS
            for kk in range(MT):
                nc.sync.dma_start(jrr[kk][:], jt_in[base + kk * 128: base + (kk + 1) * 128, :])
            for m in range(MT):
                msl = slice(m * 128, (m + 1) * 128)
                for hf in range(ECHUNKS):
                    h0 = hf * ECH
                    psum = pps.tile([128, ECH], mybir.dt.float32, name="ps", tag="ps")
                    for kk in range(MT):
                        for c in range(ECH // NCH):
                            nc.tensor.matmul(
                                psum[:, c * NCH:(c + 1) * NCH],
                                jrr[kk][:, msl],
                                xc[kk][:, h0 + c * NCH:h0 + (c + 1) * NCH],
                                start=(kk == 0), stop=False,
                            )
                    for c in range(ECH // NCH):
                        # bias: psum += h[m-block]^T @ (A_k broadcast row)
                        nc.tensor.matmul(
                            psum[:, c * NCH:(c + 1) * NCH],
                            hrow[0:1, msl],
                            av[0:1, k:k + 1].to_broadcast([1, NCH]),
                            start=False, stop=(c == ECH // NCH - 1),
                        )
                    cs = slice(h0, h0 + ECH)
                    vt = ptmp.tile([128, ECH], F32, name="vt", tag="vt")
                    sa = ptmp3.tile([128, ECH], F32, name="sa", tag="sa")
                    sb_ = ptmp3.tile([128, ECH], F32, name="sb_", tag="sb_")
                    # v = G*w + psum
                    nc.vector.scalar_tensor_tensor(vt[:], wt[m][:, cs], Gk,
                                                   psum[:], AL.mult, AL.add)
                    # x' = silu(v+1) - silu(v-1) - 1  -> f32r state
                    nc.scalar.activation(sa[:], vt[:], AF.Silu, bias=1.0, scale=1.0)
                    nc.scalar.activation(sb_[:], vt[:], AF.Silu, bias=cm1[:], scale=1.0)
                    nc.vector.scalar_tensor_tensor(xn[m][:, cs], sa[:], 1.0, sb_[:],
                                                   AL.subtract, AL.subtract)
                    # t = tanh(-50|x'| + 50.5)
                    nc.vector.tensor_scalar(sa[:].bitcast(I32), xn[m][:, cs].bitcast(I32),
                                            0x7FFFFFFF, None, AL.bitwise_and)
                    nc.gpsimd.tensor_tensor(sb_[:], vt[:], xc[m][:, cs].bitcast(F32),
                                            AL.subtract)
                    nc.scalar.activation(sa[:], sa[:], AF.Tanh, bias=c505[:], scale=-50.0)
                    # w' = (1+t) * (v - x)
                    nc.vector.scalar_tensor_tensor(wt[m][:, cs], sa[:], 1.0, sb_[:],
                                                   AL.add, AL.mult)

        xf = xa if T % 2 == 0 else xb
        for m in range(MT):
            nc.sync.dma_start(xout[m * 128:(m + 1) * 128, :], xf[m][:].bitcast(F32))

    nc.compile()
    return nc


def _host_precompute(H_real, H_imag, y_real, y_imag, delta, eta, lam, nbps):
    Hr = np.asarray(H_real, np.float64)
    Hi = np.asarray(H_imag, np.float64)
    yr = np.asarray(y_real, np.float64)
    yi = np.asarray(y_imag, np.float64)
    d = np.asarray(delta, np.float64)
    eta_s = float(np.asarray(eta).reshape(-1)[0])
    lam_s = float(np.asarray(lam).reshape(-1)[0])
    nbps = int(nbps)
    M = 2 ** nbps
    Nr, Nt = Hr.shape
    N = 2 * Nt
    rb = nbps // 2
    qam_var = 2.0 * (M - 1) / 3.0
    I = np.eye(N)
    powers = 2.0 ** (rb - 1 - np.arange(rb))
    Tm = (powers[:, None, None] * I[None, :, :]).reshape(-1, N).T
    H_t = np.block([[Hr, -Hi], [Hi, Hr]])
    y_t = np.concatenate([yr, yi], axis=0)
    U = np.linalg.inv(H_t @ H_t.T + lam_s * I) / lam_s
    HT = H_t @ Tm
    J = -(HT.T @ U @ HT) * (2.0 / qam_var)
    J = J * (1.0 - np.eye(J.shape[0]))
    z = (y_t - HT @ np.ones((N * rb, 1)) + (math.sqrt(M) - 1.0) * (H_t @ np.ones((N, 1)))) / math.sqrt(qam_var)
    h = (2.0 * (HT.T @ (U @ z)))[:, 0]
    T = d.shape[0]
    Ns = J.shape[0]
    a = np.linspace(0.0, 1.0, T)
    c0 = 2.0 * math.sqrt((Ns - 1) / float(np.sum(J * J)))
    # x_pre = B_k x + A_k (Jx) + A_k h + (d_k gamma_k) W
    A = [float(d[k] * d[k] * eta_s * c0) for k in range(T)]
    B = [float(1.0 - d[k] * d[k] * (1.0 - a[k])) for k in range(T)]
    G = [float(d[0])] + [float(d[k] / (2.0 * d[k - 1])) for k in range(1, T)]
    return J, h, A, B, G, T


def kernel(H_real, H_imag, y_real, y_imag, delta, eta, lam, x0, y0, nbps, _T=None):
    J, h, A, B, G, T = _host_precompute(
        H_real, H_imag, y_real, y_imag, delta, eta, lam, nbps)
    if _T is not None:
        T = _T
    X0 = (0.02 * (np.asarray(x0, np.float64) - 0.5)).astype(np.float32)
    W0 = (0.02 * (np.asarray(y0, np.float64) - 0.5)).astype(np.float32)

    # stacked folded matrices: J~_k = A_k*J + B_k*I  (T*NS x NS, fp32)
    J32 = J.astype(np.float32)
    JT = np.empty((T * NS, NS), np.float32)
    eye = np.eye(NS, dtype=np.float32)
    for k in range(T):
        JT[k * NS:(k + 1) * NS] = np.float32(A[k]) * J32 + np.float32(B[k]) * eye
    hrow = h.astype(np.float32).reshape(1, NS)
    avals = np.asarray(A, np.float32).reshape(1, -1)

    nc = _build_nc(G[:T], T)
    in_maps = []
    for i in range(NCORES):
        s = slice(i * BP, (i + 1) * BP)
        in_maps.append({"jt": JT, "hrow": hrow, "avals": avals[:, :T],
                        "x0": np.ascontiguousarray(X0[:, s]),
                        "w0": np.ascontiguousarray(W0[:, s])})
    res = run_bass_kernel_spmd(nc, in_maps, list(range(NCORES)))
    global LAST_RESULTS
    LAST_RESULTS = res
    out = np.concatenate([res.results[i]["xout"] for i in range(NCORES)], axis=1)
    return np.ascontiguousarray(out.T)


LAST_RESULTS = None
